# revision 23
# baseline (speedup 1.0000x reference)
"""Trainium2 Bass kernel for batched weighted complex Gram matrices.

Reference (per batch b, R/I = input_real/imag[b] (S=1024, D=256), w>=0):
    out_r = R^T diag(w) R + I^T diag(w) I      (symmetric)
    out_i = I^T diag(w) R - R^T diag(w) I      (antisymmetric)

Key algebra: with M = [R | I] (S x 2D) and m = diag(sqrt(w)) M, the Gram
G = m^T m (512x512, symmetric) contains everything:
    G = [[P, Y],[Y^T, Q]],  P = R^T W R, Q = I^T W I, Y = R^T W I
    out_r = P + Q            out_i = Y^T - Y
Per 128-chunk of the contraction, 4 PE matmuls (moving widths
512/256/384/256) accumulate: pA = [P00+Q00, P01+Q01, Y00, Y01]
(Q-blocks PE-accumulated in place), pB = [P11, Y10, Y11],
pCD = [Q10, Q11]. 1408 moving rows per chunk vs 2048 naive (-31%).

sqrt(w) is folded into the tiny [128, 32] weight tensor on the host;
on-device prep is ONE 512-col scaled copy per chunk (f32 -> f32r),
alternating between the ACT and DVE engines.

Pipeline (batch-granular software pipeline over in-order engine queues;
epilogue of batch b is spread over iterations b+1 / b+2 so it never
blocks preps, and PE transposes of batch b-1 run right after batch b's
matmuls to keep the PE saturated at its ramped clock):
    iter b: preps(b) | precopy(b-1) | mms(b), T(b-1) | finish(b-2)

Sharding: data-parallel over batch, 4 batches per core x 8 cores.
Layout: s = p*8 + c so every DMA descriptor is a contiguous 8KB run per
partition. All input DMAs are issued up-front on the sync ring; outputs
(fp16, host casts to f32) leave per batch.
"""

import sys

if "/opt/trn_rl_repo" not in sys.path:
    sys.path.insert(0, "/opt/trn_rl_repo")

import numpy as np

B, S, D = 32, 1024, 256
NCORES = 8
NB = B // NCORES          # batches per core
NCH = S // 128            # contraction chunks per batch

# tunables
WARMUP_MMS = 8            # dummy matmuls to pre-warm the PE p-state
B0_PIECES = [(0, 2), (2, 8)]          # batch-0 input DMA pieces (chunks)
B1_PIECES = [(0, 4), (4, 8)]          # batch-1 input DMA pieces
BN_PIECES = [(0, 8)]                  # batch 2..3 input DMA pieces
OUT_RING = "gpsimd"       # engine ring for output DMAs

_compiled = {}


def _build():
    import concourse.bacc as bacc
    import concourse.tile as tile
    import concourse.mybir as mybir

    f32 = mybir.dt.float32
    f32r = mybir.dt.float32r
    f16 = mybir.dt.float16

    nc = bacc.Bacc("TRN2", target_bir_lowering=False, debug=False)
    r_d = nc.dram_tensor("r", [NB, S, D], f32, kind="ExternalInput")
    i_d = nc.dram_tensor("i", [NB, S, D], f32, kind="ExternalInput")
    # host-pretransposed sqrt-weights: wsq[p, b*NCH+c] = sqrt(w[b, p*NCH+c])
    wsq_d = nc.dram_tensor("wsq", [128, NB * NCH], f32, kind="ExternalInput")
    # outputs as [b, p, a, d]; host maps (a, p) -> row a*128+p and casts to f32
    or_d = nc.dram_tensor("o_r", [NB, 128, 2, 256], f16, kind="ExternalOutput")
    oi_d = nc.dram_tensor("o_i", [NB, 128, 2, 256], f16, kind="ExternalOutput")

    out_dma = {
        "scalar": lambda *a: nc.scalar.dma_start(*a),
        "vector": lambda *a: nc.vector.dma_start(*a),
        "gpsimd": lambda *a: nc.gpsimd.dma_start(*a),
        "sync": lambda *a: nc.sync.dma_start(*a),
    }[OUT_RING]

    with tile.TileContext(nc) as tc:
        with (
            tc.tile_pool(name="wpool", bufs=1) as wpool,
            tc.tile_pool(name="xp", bufs=4) as xp,
            tc.tile_pool(name="mp", bufs=2) as mp,
            tc.tile_pool(name="yp", bufs=3) as yp,
            tc.tile_pool(name="op", bufs=3) as op,
            tc.tile_pool(name="ps", bufs=2, space="PSUM") as ps,
        ):
            w_sc = wpool.tile([128, NB * NCH], f32)
            warm = wpool.tile([128, 1], f32)
            ident = wpool.tile([128, 128], f16)
            ident32 = wpool.tile([128, 128], f32)
            x = [xp.tile([128, 2, NCH, 256], f32, name="x") for _ in range(NB)]

            # s = p*NCH + c  =>  per-partition contiguous rows in DRAM
            ir_re = i_d.rearrange("b (p c) d -> b p c d", p=128)
            rr_re = r_d.rearrange("b (p c) d -> b p c d", p=128)

            # --- issue ALL input DMAs up-front, in consumption order ---
            # The 16 DMA queues are shared FIFOs: descriptor push order IS
            # the completion order, so sequence the pieces so each chunk's
            # completion semaphore fires just before the PE needs it.
            # x[b][:, 0] = R chunks, x[b][:, 1] = I chunks (8KB runs each).
            nc.sync.dma_start(w_sc[:], wsq_d[:])
            for b in range(NB):
                pieces = (B0_PIECES, B1_PIECES, BN_PIECES, BN_PIECES)[b]
                for c0, c1 in pieces:
                    nc.sync.dma_start(x[b][:, 0, c0:c1, :], rr_re[b, :, c0:c1, :])
                    nc.sync.dma_start(x[b][:, 1, c0:c1, :], ir_re[b, :, c0:c1, :])

            nc.vector.memset(warm[:], 0.0)
            nc.scalar.copy(warm[:], warm[:])  # prime ACT table load early

            nc.vector.memset(ident32[:], 1.0)
            nc.gpsimd.affine_select(
                out=ident32[:],
                in_=ident32[:],
                compare_op=mybir.AluOpType.is_equal,
                fill=0.0,
                base=0,
                pattern=[[-1, 128]],
                channel_multiplier=1,
            )
            nc.vector.tensor_copy(ident[:], ident32[:])

            if WARMUP_MMS:
                wz = wpool.tile([128, 256], f32)
                nc.vector.memset(wz[:], 0.0)
                # warmups borrow the pCD rotation's first buffer (no extra bank)
                pwarm = ps.tile([128, 256], f32, name="pCD")
                for _ in range(WARMUP_MMS):
                    nc.tensor.matmul(
                        pwarm[:],
                        wz[:, 0:128].bitcast(f32r),
                        wz[:].bitcast(f32r),
                        start=True, stop=True, skip_group_check=True,
                    )

            state = {}

            def emit_preps(b):
                wm = mp.tile([128, NCH, 2, 256], f32r, name="wm")
                state[b] = {"wm": wm}
                for c in range(NCH):
                    col = b * NCH + c
                    nc.vector.tensor_scalar_mul(
                        wm[:, c, :, :], x[b][:, :, c, :], w_sc[:, col:col + 1]
                    )

            def emit_mms(b):
                st = state[b]
                wm = st["wm"]
                wmf = wm[:].rearrange("p c t d -> p c (t d)")
                st["ps"] = (
                    ps.tile([128, 512], f32, name="pA"),
                    ps.tile([128, 384], f32, name="pB"),
                    ps.tile([128, 256], f32, name="pCD"),
                )
                pA, pB, pCD = st["ps"]
                for c in range(NCH):
                    s0, sp = (c == 0), (c == NCH - 1)
                    nc.tensor.matmul(
                        pA[:], wmf[:, c, 0:128], wmf[:, c, 0:512],
                        start=s0, stop=False, skip_group_check=True,
                    )
                    nc.tensor.matmul(
                        pA[:, 0:256], wmf[:, c, 256:384], wmf[:, c, 256:512],
                        start=False, stop=sp, skip_group_check=True,
                    )
                    nc.tensor.matmul(
                        pB[:], wmf[:, c, 128:256], wmf[:, c, 128:512],
                        start=s0, stop=sp, skip_group_check=True,
                    )
                    nc.tensor.matmul(
                        pCD[:], wmf[:, c, 384:512], wmf[:, c, 256:512],
                        start=s0, stop=sp, skip_group_check=True,
                    )

            def emit_precopy(b):
                # ACT only: PSUM -> SBUF fp16 staging (no tensor_tensor here)
                st = state[b]
                pA, pB, pCD = st["ps"]
                y_sb = st["y"] = yp.tile([128, 4, 128], f16, name="y_sb")
                q_sb = st["q"] = yp.tile([128, 128], f16, name="q_sb")
                p11_sb = st["p11"] = yp.tile([128, 128], f16, name="p11_sb")
                or_sb = st["or"] = op.tile([128, 2, 256], f16, name="or_sb")
                st["oi"] = op.tile([128, 2, 256], f16, name="oi_sb")
                # out_r row-block 0 is ready in PSUM: [r00 r01]
                nc.scalar.copy(or_sb[:, 0, :], pA[:, 0:256])
                nc.scalar.copy(y_sb[:, 0, :], pA[:, 256:384])   # Y00
                nc.scalar.copy(y_sb[:, 1, :], pA[:, 384:512])   # Y01
                nc.scalar.copy(y_sb[:, 2, :], pB[:, 128:256])   # Y10
                nc.scalar.copy(y_sb[:, 3, :], pB[:, 256:384])   # Y11
                nc.scalar.copy(q_sb[:], pCD[:, 128:256])        # Q11
                nc.scalar.copy(p11_sb[:], pB[:, 0:128])         # P11

            def emit_T(b):
                st = state[b]
                y_sb, or_sb = st["y"], st["or"]
                pT = st["pT"] = ps.tile([128, 640], f16, name="pT", bufs=2)
                nc.tensor.transpose(pT[:, 0:128], y_sb[:, 0, :], ident[:])
                nc.tensor.transpose(pT[:, 128:256], y_sb[:, 1, :], ident[:])
                nc.tensor.transpose(pT[:, 256:384], y_sb[:, 2, :], ident[:])
                nc.tensor.transpose(pT[:, 384:512], y_sb[:, 3, :], ident[:])
                nc.tensor.transpose(pT[:, 512:640], or_sb[:, 0, 128:256], ident[:])
                # ACT: stage the transposed blocks back to SBUF
                pt_sb = st["pt_sb"] = yp.tile([128, 640], f16, name="pt_sb")
                nc.scalar.copy(pt_sb[:], pT[:])

            def emit_finish(b):
                # GpSimd only: SBUF-to-SBUF combines + output DMA issues
                st = state[b]
                y_sb, or_sb, oi_sb = st["y"], st["or"], st["oi"]
                pt_sb, q_sb, p11_sb = st["pt_sb"], st["q"], st["p11"]
                # out_i = Y^T - Y
                nc.gpsimd.tensor_sub(oi_sb[:, 0, 0:128], pt_sb[:, 0:128], y_sb[:, 0, :])
                nc.gpsimd.tensor_sub(oi_sb[:, 0, 128:256], pt_sb[:, 256:384], y_sb[:, 1, :])
                nc.gpsimd.tensor_sub(oi_sb[:, 1, 0:128], pt_sb[:, 128:256], y_sb[:, 2, :])
                nc.gpsimd.tensor_sub(oi_sb[:, 1, 128:256], pt_sb[:, 384:512], y_sb[:, 3, :])
                # r11 = P11 + Q11; r10 = r01^T
                nc.gpsimd.tensor_add(or_sb[:, 1, 128:256], p11_sb[:], q_sb[:])
                nc.gpsimd.tensor_copy(or_sb[:, 1, 0:128], pt_sb[:, 512:640])
                out_dma(or_d[b], or_sb[:])
                out_dma(oi_d[b], oi_sb[:])

            for b in range(NB):
                emit_preps(b)
                if b >= 1:
                    emit_precopy(b - 1)
                emit_mms(b)
                if b >= 1:
                    emit_T(b - 1)
                if b >= 2:
                    emit_finish(b - 2)
            emit_precopy(NB - 1)
            emit_T(NB - 1)
            emit_finish(NB - 2)
            emit_finish(NB - 1)

    nc.compile()
    return nc


def _get_nc():
    if "nc" not in _compiled:
        _compiled["nc"] = _build()
    return _compiled["nc"]


def run(input_real, input_imag, weights, trace=False):
    from concourse.bass_utils import run_bass_kernel_spmd

    nc = _get_nc()
    w = np.sqrt(np.asarray(weights, dtype=np.float64)).astype(np.float32)
    in_maps = []
    for c in range(NCORES):
        sl = slice(NB * c, NB * (c + 1))
        # wsq[p, b*NCH+ch] = sqrt(w)[b, p*NCH+ch]   (s = p*NCH + ch)
        wsq = np.ascontiguousarray(
            w[sl].reshape(NB, 128, NCH).transpose(1, 0, 2).reshape(128, NB * NCH)
        )
        in_maps.append(
            {
                "r": np.ascontiguousarray(input_real[sl], dtype=np.float32),
                "i": np.ascontiguousarray(input_imag[sl], dtype=np.float32),
                "wsq": wsq,
            }
        )
    res = run_bass_kernel_spmd(
        nc, in_maps, core_ids=list(range(NCORES)), trace=trace
    )

    def assemble(key):
        # [NB, 128, 2, 256] f16 per core -> [B, 256, 256] f32
        parts = []
        for c in range(NCORES):
            arr = np.asarray(res.results[c][key])
            parts.append(
                arr.transpose(0, 2, 1, 3).reshape(NB, 256, 256).astype(np.float32)
            )
        return np.concatenate(parts, axis=0)

    return (assemble("o_r"), assemble("o_i")), res


def kernel(input_real, input_imag, weights):
    (out_r, out_i), _ = run(input_real, input_imag, weights, trace=False)
    return (out_r, out_i)


# revision 29
# speedup vs baseline: 1.1034x; 1.1034x over previous
"""Trainium2 Bass kernel for batched weighted complex Gram matrices.

Reference (per batch b, R/I = input_real/imag[b] (S=1024, D=256), w>=0):
    out_r = R^T diag(w) R + I^T diag(w) I      (symmetric)
    out_i = I^T diag(w) R - R^T diag(w) I      (antisymmetric)

Key algebra: with M = [R | I] (S x 2D) and m = diag(sqrt(w)) M, the Gram
G = m^T m (512x512, symmetric) contains everything:
    G = [[P, Y],[Y^T, Q]],  P = R^T W R, Q = I^T W I, Y = R^T W I
    out_r = P + Q            out_i = Y^T - Y
Per 128-chunk of the contraction, 4 PE matmuls (moving widths
512/256/384/256) accumulate: pA = [P00+Q00, P01+Q01, Y00, Y01]
(Q-blocks PE-accumulated in place), pB = [P11, Y10, Y11],
pCD = [Q10, Q11]. 1408 moving rows per chunk vs 2048 naive (-31%).

sqrt(w) is folded into the tiny [128, 32] weight tensor on the host;
on-device prep is ONE 512-col scaled copy per chunk (f32 -> f32r),
alternating between the ACT and DVE engines.

Pipeline (batch-granular software pipeline over in-order engine queues;
epilogue of batch b is spread over iterations b+1 / b+2 so it never
blocks preps, and PE transposes of batch b-1 run right after batch b's
matmuls to keep the PE saturated at its ramped clock):
    iter b: preps(b) | precopy(b-1) | mms(b), T(b-1) | finish(b-2)

Sharding: data-parallel over batch, 4 batches per core x 8 cores.
Layout: s = p*8 + c so every DMA descriptor is a contiguous 8KB run per
partition. All input DMAs are issued up-front on the sync ring; outputs
(fp16, host casts to f32) leave per batch.
"""

import sys

if "/opt/trn_rl_repo" not in sys.path:
    sys.path.insert(0, "/opt/trn_rl_repo")

import numpy as np

B, S, D = 32, 1024, 256
NCORES = 8
NB = B // NCORES          # batches per core
NCH = S // 128            # contraction chunks per batch

# tunables
WARMUP_MMS = 8            # dummy matmuls to pre-warm the PE p-state
B0_PIECES = [(0, 2), (2, 8)]          # batch-0 input DMA pieces (chunks)
BN_PIECES = [(0, 8)]                  # mid-stream batches: single DMA
BL_PIECES = [(0, 4), (4, 8)]          # last batch: halves for a short tail
FILLER_MMS = 10                       # junk matmuls between batches (hold HAM)
OUT_RING = "gpsimd"       # engine ring for output DMAs

_compiled = {}


def _build():
    import concourse.bacc as bacc
    import concourse.tile as tile
    import concourse.mybir as mybir

    f32 = mybir.dt.float32
    f32r = mybir.dt.float32r
    f16 = mybir.dt.float16

    nc = bacc.Bacc("TRN2", target_bir_lowering=False, debug=False)
    r_d = nc.dram_tensor("r", [NB, S, D], f32, kind="ExternalInput")
    i_d = nc.dram_tensor("i", [NB, S, D], f32, kind="ExternalInput")
    # host-pretransposed sqrt-weights: wsq[p, b*NCH+c] = sqrt(w[b, p*NCH+c])
    wsq_d = nc.dram_tensor("wsq", [128, NB * NCH], f32, kind="ExternalInput")
    # outputs as [b, p, a, d]; host maps (a, p) -> row a*128+p and casts to f32
    or_d = nc.dram_tensor("o_r", [NB, 128, 2, 256], f16, kind="ExternalOutput")
    oi_d = nc.dram_tensor("o_i", [NB, 128, 2, 256], f16, kind="ExternalOutput")

    out_dma = {
        "scalar": lambda *a: nc.scalar.dma_start(*a),
        "vector": lambda *a: nc.vector.dma_start(*a),
        "gpsimd": lambda *a: nc.gpsimd.dma_start(*a),
        "sync": lambda *a: nc.sync.dma_start(*a),
    }[OUT_RING]

    with tile.TileContext(nc) as tc:
        with (
            tc.tile_pool(name="wpool", bufs=1) as wpool,
            tc.tile_pool(name="xp", bufs=4) as xp,
            tc.tile_pool(name="mp", bufs=2) as mp,
            tc.tile_pool(name="yp", bufs=3) as yp,
            tc.tile_pool(name="op", bufs=3) as op,
            tc.tile_pool(name="ps", bufs=2, space="PSUM") as ps,
        ):
            w_sc = wpool.tile([128, NB * NCH], f32)
            warm = wpool.tile([128, 1], f32)
            ident = wpool.tile([128, 128], f16)
            ident32 = wpool.tile([128, 128], f32)
            x = [xp.tile([128, 2, NCH, 256], f32, name="x") for _ in range(NB)]

            # s = p*NCH + c  =>  per-partition contiguous rows in DRAM
            ir_re = i_d.rearrange("b (p c) d -> b p c d", p=128)
            rr_re = r_d.rearrange("b (p c) d -> b p c d", p=128)

            # --- issue ALL input DMAs up-front, in consumption order ---
            # The 16 DMA queues are shared FIFOs: descriptor push order IS
            # the completion order, so sequence the pieces so each chunk's
            # completion semaphore fires just before the PE needs it.
            # x[b][:, 0] = R chunks, x[b][:, 1] = I chunks (8KB runs each).
            nc.sync.dma_start(w_sc[:], wsq_d[:])
            for b in range(NB):
                pieces = (B0_PIECES, BN_PIECES, BN_PIECES, BL_PIECES)[b]
                for c0, c1 in pieces:
                    nc.sync.dma_start(x[b][:, 0, c0:c1, :], rr_re[b, :, c0:c1, :])
                    nc.sync.dma_start(x[b][:, 1, c0:c1, :], ir_re[b, :, c0:c1, :])

            nc.vector.memset(warm[:], 0.0)
            nc.scalar.copy(warm[:], warm[:])  # prime ACT table load early

            nc.vector.memset(ident32[:], 1.0)
            nc.gpsimd.affine_select(
                out=ident32[:],
                in_=ident32[:],
                compare_op=mybir.AluOpType.is_equal,
                fill=0.0,
                base=0,
                pattern=[[-1, 128]],
                channel_multiplier=1,
            )
            nc.vector.tensor_copy(ident[:], ident32[:])

            wz = wpool.tile([128, 256], f32)
            nc.vector.memset(wz[:], 0.0)
            pwarm = ps.tile([128, 256], f32, name="pwarm", bufs=1)
            for _ in range(WARMUP_MMS):
                nc.tensor.matmul(
                    pwarm[:],
                    wz[:, 0:128].bitcast(f32r),
                    wz[:].bitcast(f32r),
                    start=True, stop=True, skip_group_check=True,
                )

            state = {}

            def emit_preps(b):
                wm = mp.tile([128, NCH, 2, 256], f32r, name="wm")
                state[b] = {"wm": wm}
                for c in range(NCH):
                    col = b * NCH + c
                    nc.vector.tensor_scalar_mul(
                        wm[:, c, :, :], x[b][:, :, c, :], w_sc[:, col:col + 1]
                    )

            def emit_mms(b):
                st = state[b]
                wm = st["wm"]
                wmf = wm[:].rearrange("p c t d -> p c (t d)")
                st["ps"] = (
                    ps.tile([128, 512], f32, name="pA"),
                    ps.tile([128, 384], f32, name="pB"),
                    ps.tile([128, 256], f32, name="pCD"),
                )
                pA, pB, pCD = st["ps"]
                for c in range(NCH):
                    s0, sp = (c == 0), (c == NCH - 1)
                    nc.tensor.matmul(
                        pA[:], wmf[:, c, 0:128], wmf[:, c, 0:512],
                        start=s0, stop=False, skip_group_check=True,
                    )
                    nc.tensor.matmul(
                        pA[:, 0:256], wmf[:, c, 256:384], wmf[:, c, 256:512],
                        start=False, stop=sp, skip_group_check=True,
                    )
                    nc.tensor.matmul(
                        pB[:], wmf[:, c, 128:256], wmf[:, c, 128:512],
                        start=s0, stop=sp, skip_group_check=True,
                    )
                    nc.tensor.matmul(
                        pCD[:], wmf[:, c, 384:512], wmf[:, c, 256:512],
                        start=s0, stop=sp, skip_group_check=True,
                    )

            def emit_precopy(b):
                # ACT only: PSUM -> SBUF fp16 staging (no tensor_tensor here)
                st = state[b]
                pA, pB, pCD = st["ps"]
                y_sb = st["y"] = yp.tile([128, 4, 128], f16, name="y_sb")
                q_sb = st["q"] = yp.tile([128, 128], f16, name="q_sb")
                p11_sb = st["p11"] = yp.tile([128, 128], f16, name="p11_sb")
                or_sb = st["or"] = op.tile([128, 2, 256], f16, name="or_sb")
                st["oi"] = op.tile([128, 2, 256], f16, name="oi_sb")
                # out_r row-block 0 is ready in PSUM: [r00 r01]
                nc.scalar.copy(or_sb[:, 0, :], pA[:, 0:256])
                nc.scalar.copy(y_sb[:, 0, :], pA[:, 256:384])   # Y00
                nc.scalar.copy(y_sb[:, 1, :], pA[:, 384:512])   # Y01
                nc.scalar.copy(y_sb[:, 2, :], pB[:, 128:256])   # Y10
                nc.scalar.copy(y_sb[:, 3, :], pB[:, 256:384])   # Y11
                nc.scalar.copy(q_sb[:], pCD[:, 128:256])        # Q11
                nc.scalar.copy(p11_sb[:], pB[:, 0:128])         # P11

            def emit_T(b):
                st = state[b]
                y_sb, or_sb = st["y"], st["or"]
                pT = st["pT"] = ps.tile([128, 640], f16, name="pT", bufs=1)
                nc.tensor.transpose(pT[:, 0:128], y_sb[:, 0, :], ident[:])
                nc.tensor.transpose(pT[:, 128:256], y_sb[:, 1, :], ident[:])
                nc.tensor.transpose(pT[:, 256:384], y_sb[:, 2, :], ident[:])
                nc.tensor.transpose(pT[:, 384:512], y_sb[:, 3, :], ident[:])
                nc.tensor.transpose(pT[:, 512:640], or_sb[:, 0, 128:256], ident[:])
                # ACT: stage the transposed blocks back to SBUF
                pt_sb = st["pt_sb"] = yp.tile([128, 640], f16, name="pt_sb")
                nc.scalar.copy(pt_sb[:], pT[:])

            def emit_finish(b, eng, ring):
                # SBUF-to-SBUF combines + output DMA issues. GpSimd for
                # mid-stream batches (keeps DVE/ACT queues clean); DVE for
                # the last batch (fast tail).
                st = state[b]
                y_sb, or_sb, oi_sb = st["y"], st["or"], st["oi"]
                pt_sb, q_sb, p11_sb = st["pt_sb"], st["q"], st["p11"]
                # out_i = Y^T - Y
                eng.tensor_sub(oi_sb[:, 0, 0:128], pt_sb[:, 0:128], y_sb[:, 0, :])
                eng.tensor_sub(oi_sb[:, 0, 128:256], pt_sb[:, 256:384], y_sb[:, 1, :])
                eng.tensor_sub(oi_sb[:, 1, 0:128], pt_sb[:, 128:256], y_sb[:, 2, :])
                eng.tensor_sub(oi_sb[:, 1, 128:256], pt_sb[:, 384:512], y_sb[:, 3, :])
                # r11 = P11 + Q11; r10 = r01^T
                eng.tensor_add(or_sb[:, 1, 128:256], p11_sb[:], q_sb[:])
                eng.tensor_copy(or_sb[:, 1, 0:128], pt_sb[:, 512:640])
                ring(or_d[b], or_sb[:])
                ring(oi_d[b], oi_sb[:])

            def emit_fillers(n):
                for _ in range(n):
                    nc.tensor.matmul(
                        pwarm[:],
                        wz[:, 0:128].bitcast(f32r),
                        wz[:].bitcast(f32r),
                        start=True, stop=True, skip_group_check=True,
                    )

            for b in range(NB):
                emit_preps(b)
                if b >= 1:
                    emit_precopy(b - 1)
                emit_mms(b)
                if b >= 1:
                    emit_T(b - 1)
                if b < NB - 1:
                    emit_fillers(FILLER_MMS)
                if b >= 2:
                    emit_finish(b - 2, nc.gpsimd, out_dma)
            emit_precopy(NB - 1)
            emit_T(NB - 1)
            emit_finish(NB - 2, nc.gpsimd, out_dma)
            emit_finish(NB - 1, nc.vector, lambda *a: nc.scalar.dma_start(*a))

    nc.compile()
    return nc


def _get_nc():
    if "nc" not in _compiled:
        _compiled["nc"] = _build()
    return _compiled["nc"]


def run(input_real, input_imag, weights, trace=False):
    from concourse.bass_utils import run_bass_kernel_spmd

    nc = _get_nc()
    w = np.sqrt(np.asarray(weights, dtype=np.float64)).astype(np.float32)
    in_maps = []
    for c in range(NCORES):
        sl = slice(NB * c, NB * (c + 1))
        # wsq[p, b*NCH+ch] = sqrt(w)[b, p*NCH+ch]   (s = p*NCH + ch)
        wsq = np.ascontiguousarray(
            w[sl].reshape(NB, 128, NCH).transpose(1, 0, 2).reshape(128, NB * NCH)
        )
        in_maps.append(
            {
                "r": np.ascontiguousarray(input_real[sl], dtype=np.float32),
                "i": np.ascontiguousarray(input_imag[sl], dtype=np.float32),
                "wsq": wsq,
            }
        )
    res = run_bass_kernel_spmd(
        nc, in_maps, core_ids=list(range(NCORES)), trace=trace
    )

    def assemble(key):
        # [NB, 128, 2, 256] f16 per core -> [B, 256, 256] f32
        parts = []
        for c in range(NCORES):
            arr = np.asarray(res.results[c][key])
            parts.append(
                arr.transpose(0, 2, 1, 3).reshape(NB, 256, 256).astype(np.float32)
            )
        return np.concatenate(parts, axis=0)

    return (assemble("o_r"), assemble("o_i")), res


def kernel(input_real, input_imag, weights):
    (out_r, out_i), _ = run(input_real, input_imag, weights, trace=False)
    return (out_r, out_i)


# revision 30
# speedup vs baseline: 1.1882x; 1.0769x over previous
"""Trainium2 Bass kernel for batched weighted complex Gram matrices.

Reference (per batch b, R/I = input_real/imag[b] (S=1024, D=256), w>=0):
    out_r = R^T diag(w) R + I^T diag(w) I      (symmetric)
    out_i = I^T diag(w) R - R^T diag(w) I      (antisymmetric)

Key algebra: with M = [R | I] (S x 2D) and m = diag(sqrt(w)) M, the Gram
G = m^T m (512x512, symmetric) contains everything:
    G = [[P, Y],[Y^T, Q]],  P = R^T W R, Q = I^T W I, Y = R^T W I
    out_r = P + Q            out_i = Y^T - Y
Per 128-chunk of the contraction, 4 PE matmuls (moving widths
512/256/384/256) accumulate: pA = [P00+Q00, P01+Q01, Y00, Y01]
(Q-blocks PE-accumulated in place), pB = [P11, Y10, Y11],
pCD = [Q10, Q11]. 1408 moving rows per chunk vs 2048 naive (-31%).

sqrt(w) is folded into the tiny [128, 32] weight tensor on the host;
on-device prep is ONE 512-col scaled copy per chunk (f32 -> f32r),
alternating between the ACT and DVE engines.

Pipeline (batch-granular software pipeline over in-order engine queues;
epilogue of batch b is spread over iterations b+1 / b+2 so it never
blocks preps, and PE transposes of batch b-1 run right after batch b's
matmuls to keep the PE saturated at its ramped clock):
    iter b: preps(b) | precopy(b-1) | mms(b), T(b-1) | finish(b-2)

Sharding: data-parallel over batch, 4 batches per core x 8 cores.
Layout: s = p*8 + c so every DMA descriptor is a contiguous 8KB run per
partition. All input DMAs are issued up-front on the sync ring; outputs
(fp16, host casts to f32) leave per batch.
"""

import sys

if "/opt/trn_rl_repo" not in sys.path:
    sys.path.insert(0, "/opt/trn_rl_repo")

import numpy as np

B, S, D = 32, 1024, 256
NCORES = 8
NB = B // NCORES          # batches per core
NCH = S // 128            # contraction chunks per batch

# tunables
WARMUP_MMS = 14           # dummy matmuls to pre-warm the PE p-state
B0_PIECES = [(0, 8)]                  # batch-0 input DMA pieces (chunks)
BN_PIECES = [(0, 8)]                  # mid-stream batches: single DMA
BL_PIECES = [(0, 4), (4, 8)]          # last batch: halves for a short tail
FILLER_MMS = 10                       # junk matmuls between batches (hold HAM)
OUT_RING = "gpsimd"       # engine ring for output DMAs

_compiled = {}


def _build():
    import concourse.bacc as bacc
    import concourse.tile as tile
    import concourse.mybir as mybir

    f32 = mybir.dt.float32
    f32r = mybir.dt.float32r
    f16 = mybir.dt.float16

    nc = bacc.Bacc("TRN2", target_bir_lowering=False, debug=False)
    r_d = nc.dram_tensor("r", [NB, S, D], f32, kind="ExternalInput")
    i_d = nc.dram_tensor("i", [NB, S, D], f32, kind="ExternalInput")
    # host-pretransposed sqrt-weights: wsq[p, b*NCH+c] = sqrt(w[b, p*NCH+c])
    wsq_d = nc.dram_tensor("wsq", [128, NB * NCH], f32, kind="ExternalInput")
    # outputs as [b, p, a, d]; host maps (a, p) -> row a*128+p and casts to f32
    or_d = nc.dram_tensor("o_r", [NB, 128, 2, 256], f16, kind="ExternalOutput")
    oi_d = nc.dram_tensor("o_i", [NB, 128, 2, 256], f16, kind="ExternalOutput")

    out_dma = {
        "scalar": lambda *a: nc.scalar.dma_start(*a),
        "vector": lambda *a: nc.vector.dma_start(*a),
        "gpsimd": lambda *a: nc.gpsimd.dma_start(*a),
        "sync": lambda *a: nc.sync.dma_start(*a),
    }[OUT_RING]

    with tile.TileContext(nc) as tc:
        with (
            tc.tile_pool(name="wpool", bufs=1) as wpool,
            tc.tile_pool(name="xp", bufs=4) as xp,
            tc.tile_pool(name="mp", bufs=2) as mp,
            tc.tile_pool(name="yp", bufs=3) as yp,
            tc.tile_pool(name="op", bufs=3) as op,
            tc.tile_pool(name="ps", bufs=2, space="PSUM") as ps,
        ):
            w_sc = wpool.tile([128, NB * NCH], f32)
            warm = wpool.tile([128, 1], f32)
            ident = wpool.tile([128, 128], f16)
            ident32 = wpool.tile([128, 128], f32)
            x = [xp.tile([128, 2, NCH, 256], f32, name="x") for _ in range(NB)]

            # s = p*NCH + c  =>  per-partition contiguous rows in DRAM
            ir_re = i_d.rearrange("b (p c) d -> b p c d", p=128)
            rr_re = r_d.rearrange("b (p c) d -> b p c d", p=128)

            # --- issue ALL input DMAs up-front, in consumption order ---
            # The 16 DMA queues are shared FIFOs: descriptor push order IS
            # the completion order, so sequence the pieces so each chunk's
            # completion semaphore fires just before the PE needs it.
            # x[b][:, 0] = R chunks, x[b][:, 1] = I chunks (8KB runs each).
            nc.sync.dma_start(w_sc[:], wsq_d[:])
            for b in range(NB):
                pieces = (B0_PIECES, BN_PIECES, BN_PIECES, BL_PIECES)[b]
                for c0, c1 in pieces:
                    nc.sync.dma_start(x[b][:, 0, c0:c1, :], rr_re[b, :, c0:c1, :])
                    nc.sync.dma_start(x[b][:, 1, c0:c1, :], ir_re[b, :, c0:c1, :])

            nc.vector.memset(warm[:], 0.0)
            nc.scalar.copy(warm[:], warm[:])  # prime ACT table load early

            nc.vector.memset(ident32[:], 1.0)
            nc.gpsimd.affine_select(
                out=ident32[:],
                in_=ident32[:],
                compare_op=mybir.AluOpType.is_equal,
                fill=0.0,
                base=0,
                pattern=[[-1, 128]],
                channel_multiplier=1,
            )
            nc.vector.tensor_copy(ident[:], ident32[:])

            wz = wpool.tile([128, 256], f32)
            nc.vector.memset(wz[:], 0.0)
            pwarm = ps.tile([128, 256], f32, name="pwarm", bufs=1)
            for _ in range(WARMUP_MMS):
                nc.tensor.matmul(
                    pwarm[:],
                    wz[:, 0:128].bitcast(f32r),
                    wz[:].bitcast(f32r),
                    start=True, stop=True, skip_group_check=True,
                )

            state = {}

            def emit_preps(b):
                wm = mp.tile([128, NCH, 2, 256], f32r, name="wm")
                state[b] = {"wm": wm}
                for c in range(NCH):
                    col = b * NCH + c
                    nc.vector.tensor_scalar_mul(
                        wm[:, c, :, :], x[b][:, :, c, :], w_sc[:, col:col + 1]
                    )

            def emit_mms(b):
                st = state[b]
                wm = st["wm"]
                wmf = wm[:].rearrange("p c t d -> p c (t d)")
                st["ps"] = (
                    ps.tile([128, 512], f32, name="pA"),
                    ps.tile([128, 384], f32, name="pB"),
                    ps.tile([128, 256], f32, name="pCD"),
                )
                pA, pB, pCD = st["ps"]
                for c in range(NCH):
                    s0, sp = (c == 0), (c == NCH - 1)
                    nc.tensor.matmul(
                        pA[:], wmf[:, c, 0:128], wmf[:, c, 0:512],
                        start=s0, stop=False, skip_group_check=True,
                    )
                    nc.tensor.matmul(
                        pA[:, 0:256], wmf[:, c, 256:384], wmf[:, c, 256:512],
                        start=False, stop=sp, skip_group_check=True,
                    )
                    nc.tensor.matmul(
                        pB[:], wmf[:, c, 128:256], wmf[:, c, 128:512],
                        start=s0, stop=sp, skip_group_check=True,
                    )
                    nc.tensor.matmul(
                        pCD[:], wmf[:, c, 384:512], wmf[:, c, 256:512],
                        start=s0, stop=sp, skip_group_check=True,
                    )

            def emit_precopy(b):
                # ACT only: PSUM -> SBUF fp16 staging (no tensor_tensor here)
                st = state[b]
                pA, pB, pCD = st["ps"]
                y_sb = st["y"] = yp.tile([128, 4, 128], f16, name="y_sb")
                q_sb = st["q"] = yp.tile([128, 128], f16, name="q_sb")
                p11_sb = st["p11"] = yp.tile([128, 128], f16, name="p11_sb")
                or_sb = st["or"] = op.tile([128, 2, 256], f16, name="or_sb")
                st["oi"] = op.tile([128, 2, 256], f16, name="oi_sb")
                # out_r row-block 0 is ready in PSUM: [r00 r01]
                nc.scalar.copy(or_sb[:, 0, :], pA[:, 0:256])
                nc.scalar.copy(y_sb[:, 0, :], pA[:, 256:384])   # Y00
                nc.scalar.copy(y_sb[:, 1, :], pA[:, 384:512])   # Y01
                nc.scalar.copy(y_sb[:, 2, :], pB[:, 128:256])   # Y10
                nc.scalar.copy(y_sb[:, 3, :], pB[:, 256:384])   # Y11
                nc.scalar.copy(q_sb[:], pCD[:, 128:256])        # Q11
                nc.scalar.copy(p11_sb[:], pB[:, 0:128])         # P11

            def emit_T(b):
                st = state[b]
                y_sb, or_sb = st["y"], st["or"]
                pT = st["pT"] = ps.tile([128, 640], f16, name="pT", bufs=1)
                nc.tensor.transpose(pT[:, 0:128], y_sb[:, 0, :], ident[:])
                nc.tensor.transpose(pT[:, 128:256], y_sb[:, 1, :], ident[:])
                nc.tensor.transpose(pT[:, 256:384], y_sb[:, 2, :], ident[:])
                nc.tensor.transpose(pT[:, 384:512], y_sb[:, 3, :], ident[:])
                nc.tensor.transpose(pT[:, 512:640], or_sb[:, 0, 128:256], ident[:])
                # ACT: stage the transposed blocks back to SBUF
                pt_sb = st["pt_sb"] = yp.tile([128, 640], f16, name="pt_sb")
                nc.scalar.copy(pt_sb[:], pT[:])

            def emit_finish(b, eng, ring):
                # SBUF-to-SBUF combines + output DMA issues. GpSimd for
                # mid-stream batches (keeps DVE/ACT queues clean); DVE for
                # the last batch (fast tail).
                st = state[b]
                y_sb, or_sb, oi_sb = st["y"], st["or"], st["oi"]
                pt_sb, q_sb, p11_sb = st["pt_sb"], st["q"], st["p11"]
                # out_i = Y^T - Y
                eng.tensor_sub(oi_sb[:, 0, 0:128], pt_sb[:, 0:128], y_sb[:, 0, :])
                eng.tensor_sub(oi_sb[:, 0, 128:256], pt_sb[:, 256:384], y_sb[:, 1, :])
                eng.tensor_sub(oi_sb[:, 1, 0:128], pt_sb[:, 128:256], y_sb[:, 2, :])
                eng.tensor_sub(oi_sb[:, 1, 128:256], pt_sb[:, 384:512], y_sb[:, 3, :])
                # r11 = P11 + Q11; r10 = r01^T
                eng.tensor_add(or_sb[:, 1, 128:256], p11_sb[:], q_sb[:])
                eng.tensor_copy(or_sb[:, 1, 0:128], pt_sb[:, 512:640])
                ring(or_d[b], or_sb[:])
                ring(oi_d[b], oi_sb[:])

            def emit_fillers(n):
                for _ in range(n):
                    nc.tensor.matmul(
                        pwarm[:],
                        wz[:, 0:128].bitcast(f32r),
                        wz[:].bitcast(f32r),
                        start=True, stop=True, skip_group_check=True,
                    )

            for b in range(NB):
                emit_preps(b)
                if b >= 1:
                    emit_precopy(b - 1)
                emit_mms(b)
                if b >= 1:
                    emit_T(b - 1)
                if b < NB - 1:
                    emit_fillers(FILLER_MMS)
                if b >= 2:
                    emit_finish(b - 2, nc.gpsimd, out_dma)
            emit_precopy(NB - 1)
            emit_T(NB - 1)
            emit_finish(NB - 2, nc.gpsimd, out_dma)
            emit_finish(NB - 1, nc.vector, lambda *a: nc.scalar.dma_start(*a))

    nc.compile()
    return nc


def _get_nc():
    if "nc" not in _compiled:
        _compiled["nc"] = _build()
    return _compiled["nc"]


def run(input_real, input_imag, weights, trace=False):
    from concourse.bass_utils import run_bass_kernel_spmd

    nc = _get_nc()
    w = np.sqrt(np.asarray(weights, dtype=np.float64)).astype(np.float32)
    in_maps = []
    for c in range(NCORES):
        sl = slice(NB * c, NB * (c + 1))
        # wsq[p, b*NCH+ch] = sqrt(w)[b, p*NCH+ch]   (s = p*NCH + ch)
        wsq = np.ascontiguousarray(
            w[sl].reshape(NB, 128, NCH).transpose(1, 0, 2).reshape(128, NB * NCH)
        )
        in_maps.append(
            {
                "r": np.ascontiguousarray(input_real[sl], dtype=np.float32),
                "i": np.ascontiguousarray(input_imag[sl], dtype=np.float32),
                "wsq": wsq,
            }
        )
    res = run_bass_kernel_spmd(
        nc, in_maps, core_ids=list(range(NCORES)), trace=trace
    )

    def assemble(key):
        # [NB, 128, 2, 256] f16 per core -> [B, 256, 256] f32
        parts = []
        for c in range(NCORES):
            arr = np.asarray(res.results[c][key])
            parts.append(
                arr.transpose(0, 2, 1, 3).reshape(NB, 256, 256).astype(np.float32)
            )
        return np.concatenate(parts, axis=0)

    return (assemble("o_r"), assemble("o_i")), res


def kernel(input_real, input_imag, weights):
    (out_r, out_i), _ = run(input_real, input_imag, weights, trace=False)
    return (out_r, out_i)


# revision 31
# speedup vs baseline: 1.2152x; 1.0227x over previous
"""Trainium2 Bass kernel for batched weighted complex Gram matrices.

Reference (per batch b, R/I = input_real/imag[b] (S=1024, D=256), w>=0):
    out_r = R^T diag(w) R + I^T diag(w) I      (symmetric)
    out_i = I^T diag(w) R - R^T diag(w) I      (antisymmetric)

Key algebra: with M = [R | I] (S x 2D) and m = diag(sqrt(w)) M, the Gram
G = m^T m (512x512, symmetric) contains everything:
    G = [[P, Y],[Y^T, Q]],  P = R^T W R, Q = I^T W I, Y = R^T W I
    out_r = P + Q            out_i = Y^T - Y
Per 128-chunk of the contraction, 4 PE matmuls (moving widths
512/256/384/256) accumulate: pA = [P00+Q00, P01+Q01, Y00, Y01]
(Q-blocks PE-accumulated in place), pB = [P11, Y10, Y11],
pCD = [Q10, Q11]. 1408 moving rows per chunk vs 2048 naive (-31%).

sqrt(w) is folded into the tiny [128, 32] weight tensor on the host;
on-device prep is ONE 512-col scaled copy per chunk (f32 -> f32r),
alternating between the ACT and DVE engines.

Pipeline (batch-granular software pipeline over in-order engine queues;
epilogue of batch b is spread over iterations b+1 / b+2 so it never
blocks preps, and PE transposes of batch b-1 run right after batch b's
matmuls to keep the PE saturated at its ramped clock):
    iter b: preps(b) | precopy(b-1) | mms(b), T(b-1) | finish(b-2)

Sharding: data-parallel over batch, 4 batches per core x 8 cores.
Layout: s = p*8 + c so every DMA descriptor is a contiguous 8KB run per
partition. All input DMAs are issued up-front on the sync ring; outputs
(fp16, host casts to f32) leave per batch.
"""

import sys

if "/opt/trn_rl_repo" not in sys.path:
    sys.path.insert(0, "/opt/trn_rl_repo")

import numpy as np

B, S, D = 32, 1024, 256
NCORES = 8
NB = B // NCORES          # batches per core
NCH = S // 128            # contraction chunks per batch

# tunables
WARMUP_MMS = 14           # dummy matmuls to pre-warm the PE p-state
B0_PIECES = [(0, 8)]                  # batch-0 input DMA pieces (chunks)
BN_PIECES = [(0, 8)]                  # mid-stream batches: single DMA
BL_PIECES = [(0, 6), (6, 8)]          # last batch: short final piece
FILLER_MMS = (10, 8, 0)               # junk matmuls after batches 0,1,2 (hold HAM)
OUT_RING = "gpsimd"       # engine ring for output DMAs

_compiled = {}


def _build():
    import concourse.bacc as bacc
    import concourse.tile as tile
    import concourse.mybir as mybir

    f32 = mybir.dt.float32
    f32r = mybir.dt.float32r
    f16 = mybir.dt.float16

    nc = bacc.Bacc("TRN2", target_bir_lowering=False, debug=False)
    r_d = nc.dram_tensor("r", [NB, S, D], f32, kind="ExternalInput")
    i_d = nc.dram_tensor("i", [NB, S, D], f32, kind="ExternalInput")
    # host-pretransposed sqrt-weights: wsq[p, b*NCH+c] = sqrt(w[b, p*NCH+c])
    wsq_d = nc.dram_tensor("wsq", [128, NB * NCH], f32, kind="ExternalInput")
    # merged output [b, p, (or0 or1 oi0 oi1), d]; host splits + casts to f32
    oo_d = nc.dram_tensor("o_o", [NB, 128, 4, 256], f16, kind="ExternalOutput")

    out_dma = {
        "scalar": lambda *a: nc.scalar.dma_start(*a),
        "vector": lambda *a: nc.vector.dma_start(*a),
        "gpsimd": lambda *a: nc.gpsimd.dma_start(*a),
        "sync": lambda *a: nc.sync.dma_start(*a),
    }[OUT_RING]

    with tile.TileContext(nc) as tc:
        with (
            tc.tile_pool(name="wpool", bufs=1) as wpool,
            tc.tile_pool(name="xp", bufs=4) as xp,
            tc.tile_pool(name="mp", bufs=2) as mp,
            tc.tile_pool(name="yp", bufs=3) as yp,
            tc.tile_pool(name="op", bufs=3) as op,
            tc.tile_pool(name="ps", bufs=2, space="PSUM") as ps,
        ):
            w_sc = wpool.tile([128, NB * NCH], f32)
            warm = wpool.tile([128, 1], f32)
            ident = wpool.tile([128, 128], f16)
            ident32 = wpool.tile([128, 128], f32)
            x = [xp.tile([128, 2, NCH, 256], f32, name="x") for _ in range(NB)]

            # s = p*NCH + c  =>  per-partition contiguous rows in DRAM
            ir_re = i_d.rearrange("b (p c) d -> b p c d", p=128)
            rr_re = r_d.rearrange("b (p c) d -> b p c d", p=128)

            # --- issue ALL input DMAs up-front, in consumption order ---
            # The 16 DMA queues are shared FIFOs: descriptor push order IS
            # the completion order, so sequence the pieces so each chunk's
            # completion semaphore fires just before the PE needs it.
            # x[b][:, 0] = R chunks, x[b][:, 1] = I chunks (8KB runs each).
            nc.sync.dma_start(w_sc[:], wsq_d[:])
            for b in range(NB):
                pieces = (B0_PIECES, BN_PIECES, BN_PIECES, BL_PIECES)[b]
                for c0, c1 in pieces:
                    nc.sync.dma_start(x[b][:, 0, c0:c1, :], rr_re[b, :, c0:c1, :])
                    nc.sync.dma_start(x[b][:, 1, c0:c1, :], ir_re[b, :, c0:c1, :])

            nc.vector.memset(warm[:], 0.0)
            nc.scalar.copy(warm[:], warm[:])  # prime ACT table load early

            nc.vector.memset(ident32[:], 1.0)
            nc.gpsimd.affine_select(
                out=ident32[:],
                in_=ident32[:],
                compare_op=mybir.AluOpType.is_equal,
                fill=0.0,
                base=0,
                pattern=[[-1, 128]],
                channel_multiplier=1,
            )
            nc.vector.tensor_copy(ident[:], ident32[:])

            wz = wpool.tile([128, 256], f32)
            nc.vector.memset(wz[:], 0.0)
            pwarm = ps.tile([128, 256], f32, name="pwarm", bufs=1)
            for _ in range(WARMUP_MMS):
                nc.tensor.matmul(
                    pwarm[:],
                    wz[:, 0:128].bitcast(f32r),
                    wz[:].bitcast(f32r),
                    start=True, stop=True, skip_group_check=True,
                )

            state = {}

            def emit_preps(b):
                wm = mp.tile([128, NCH, 2, 256], f32r, name="wm")
                state[b] = {"wm": wm}
                for c in range(NCH):
                    col = b * NCH + c
                    nc.vector.tensor_scalar_mul(
                        wm[:, c, :, :], x[b][:, :, c, :], w_sc[:, col:col + 1]
                    )

            def emit_mms(b):
                st = state[b]
                wm = st["wm"]
                wmf = wm[:].rearrange("p c t d -> p c (t d)")
                st["ps"] = (
                    ps.tile([128, 512], f32, name="pA"),
                    ps.tile([128, 384], f32, name="pB"),
                    ps.tile([128, 256], f32, name="pCD"),
                )
                pA, pB, pCD = st["ps"]
                for c in range(NCH):
                    s0, sp = (c == 0), (c == NCH - 1)
                    nc.tensor.matmul(
                        pA[:], wmf[:, c, 0:128], wmf[:, c, 0:512],
                        start=s0, stop=False, skip_group_check=True,
                    )
                    nc.tensor.matmul(
                        pA[:, 0:256], wmf[:, c, 256:384], wmf[:, c, 256:512],
                        start=False, stop=sp, skip_group_check=True,
                    )
                    nc.tensor.matmul(
                        pB[:], wmf[:, c, 128:256], wmf[:, c, 128:512],
                        start=s0, stop=sp, skip_group_check=True,
                    )
                    nc.tensor.matmul(
                        pCD[:], wmf[:, c, 384:512], wmf[:, c, 256:512],
                        start=s0, stop=sp, skip_group_check=True,
                    )

            def emit_precopy(b):
                # ACT only: PSUM -> SBUF fp16 staging (no tensor_tensor here)
                st = state[b]
                pA, pB, pCD = st["ps"]
                y_sb = st["y"] = yp.tile([128, 4, 128], f16, name="y_sb")
                q_sb = st["q"] = yp.tile([128, 128], f16, name="q_sb")
                p11_sb = st["p11"] = yp.tile([128, 128], f16, name="p11_sb")
                oo_sb = st["oo"] = op.tile([128, 4, 256], f16, name="oo_sb")
                or_sb = oo_sb[:, 0:2, :]
                # out_r row-block 0 is ready in PSUM: [r00 r01]
                nc.scalar.copy(oo_sb[:, 0, :], pA[:, 0:256])
                nc.scalar.copy(y_sb[:, 0, :], pA[:, 256:384])   # Y00
                nc.scalar.copy(y_sb[:, 1, :], pA[:, 384:512])   # Y01
                nc.scalar.copy(y_sb[:, 2, :], pB[:, 128:256])   # Y10
                nc.scalar.copy(y_sb[:, 3, :], pB[:, 256:384])   # Y11
                nc.scalar.copy(q_sb[:], pCD[:, 128:256])        # Q11
                nc.scalar.copy(p11_sb[:], pB[:, 0:128])         # P11

            def emit_T(b):
                st = state[b]
                y_sb, or_sb = st["y"], st["oo"][:, 0:2, :]
                pT = st["pT"] = ps.tile([128, 640], f16, name="pT", bufs=1)
                nc.tensor.transpose(pT[:, 0:128], y_sb[:, 0, :], ident[:])
                nc.tensor.transpose(pT[:, 128:256], y_sb[:, 1, :], ident[:])
                nc.tensor.transpose(pT[:, 256:384], y_sb[:, 2, :], ident[:])
                nc.tensor.transpose(pT[:, 384:512], y_sb[:, 3, :], ident[:])
                nc.tensor.transpose(pT[:, 512:640], or_sb[:, 0, 128:256], ident[:])
                # ACT: stage the transposed blocks back to SBUF
                pt_sb = st["pt_sb"] = yp.tile([128, 640], f16, name="pt_sb")
                nc.scalar.copy(pt_sb[:], pT[:])

            def emit_finish(b, eng, ring):
                # SBUF-to-SBUF combines + output DMA issues. GpSimd for
                # mid-stream batches (keeps DVE/ACT queues clean); DVE for
                # the last batch (fast tail).
                st = state[b]
                oo_sb = st["oo"]
                y_sb, or_sb, oi_sb = st["y"], oo_sb[:, 0:2, :], oo_sb[:, 2:4, :]
                pt_sb, q_sb, p11_sb = st["pt_sb"], st["q"], st["p11"]
                # out_i = Y^T - Y
                eng.tensor_sub(oi_sb[:, 0, 0:128], pt_sb[:, 0:128], y_sb[:, 0, :])
                eng.tensor_sub(oi_sb[:, 0, 128:256], pt_sb[:, 256:384], y_sb[:, 1, :])
                eng.tensor_sub(oi_sb[:, 1, 0:128], pt_sb[:, 128:256], y_sb[:, 2, :])
                eng.tensor_sub(oi_sb[:, 1, 128:256], pt_sb[:, 384:512], y_sb[:, 3, :])
                # r11 = P11 + Q11; r10 = r01^T
                eng.tensor_add(or_sb[:, 1, 128:256], p11_sb[:], q_sb[:])
                eng.tensor_copy(or_sb[:, 1, 0:128], pt_sb[:, 512:640])
                ring(oo_d[b], oo_sb[:])

            def emit_fillers(n):
                for _ in range(n):
                    nc.tensor.matmul(
                        pwarm[:],
                        wz[:, 0:128].bitcast(f32r),
                        wz[:].bitcast(f32r),
                        start=True, stop=True, skip_group_check=True,
                    )

            for b in range(NB):
                emit_preps(b)
                if b >= 1:
                    emit_precopy(b - 1)
                emit_mms(b)
                if b >= 1:
                    emit_T(b - 1)
                if b < NB - 1:
                    emit_fillers(FILLER_MMS[b])
                if b >= 2:
                    emit_finish(b - 2, nc.gpsimd, out_dma)
            emit_precopy(NB - 1)
            emit_T(NB - 1)
            emit_finish(NB - 2, nc.gpsimd, out_dma)
            emit_finish(NB - 1, nc.vector, lambda *a: nc.scalar.dma_start(*a))

    nc.compile()
    return nc


def _get_nc():
    if "nc" not in _compiled:
        _compiled["nc"] = _build()
    return _compiled["nc"]


def run(input_real, input_imag, weights, trace=False):
    from concourse.bass_utils import run_bass_kernel_spmd

    nc = _get_nc()
    w = np.sqrt(np.asarray(weights, dtype=np.float64)).astype(np.float32)
    in_maps = []
    for c in range(NCORES):
        sl = slice(NB * c, NB * (c + 1))
        # wsq[p, b*NCH+ch] = sqrt(w)[b, p*NCH+ch]   (s = p*NCH + ch)
        wsq = np.ascontiguousarray(
            w[sl].reshape(NB, 128, NCH).transpose(1, 0, 2).reshape(128, NB * NCH)
        )
        in_maps.append(
            {
                "r": np.ascontiguousarray(input_real[sl], dtype=np.float32),
                "i": np.ascontiguousarray(input_imag[sl], dtype=np.float32),
                "wsq": wsq,
            }
        )
    res = run_bass_kernel_spmd(
        nc, in_maps, core_ids=list(range(NCORES)), trace=trace
    )

    def assemble(half):
        # [NB, 128, 4, 256] f16 per core -> [B, 256, 256] f32
        parts = []
        for c in range(NCORES):
            arr = np.asarray(res.results[c]["o_o"])[:, :, half:half + 2, :]
            parts.append(
                arr.transpose(0, 2, 1, 3).reshape(NB, 256, 256).astype(np.float32)
            )
        return np.concatenate(parts, axis=0)

    return (assemble(0), assemble(2)), res


def kernel(input_real, input_imag, weights):
    (out_r, out_i), _ = run(input_real, input_imag, weights, trace=False)
    return (out_r, out_i)
